# revision 48
# baseline (speedup 1.0000x reference)
"""Causal multi-head attention block on 8 Trainium2 NeuronCores.

Problem: x[4,2048,1024] -> qkv proj -> 16-head causal attention -> out proj.

Sharding: 8 cores = 4 batches x 2 head-groups (8 heads each). Each core
computes, for its (batch, head-group):
  - xT (feature-on-partition) via DMA-crossbar transpose of bf16 x
  - qT/kT (feature-on-partition) and v (natural layout), all bf16
  - causal attention with scores computed transposed (scoresT[j, i]):
    fp32 PSUM scores -> exp on the Act engine (bf16 out, fully-masked
    columns skipped), causal zero-fill on GpSimd, row-sums via an
    appended ones-column on v in the attn@v matmul
  - softmax denominators inverted with reciprocal_approx_fast and
    broadcast to 64 partitions with a tiny K=2 PE matmul
  - partial out-projection with its 512 rows of W_proj
Host sums the two partials per batch and adds b_proj.

All matmuls run in bf16 (1 cycle/row on HW vs ~2 for f32r; fp32 PSUM
accumulation). The emission order software-pipelines the attention inner
loop (attn@v for tile jt-1 is emitted after scores for jt so the PE
in-order queue never head-of-line blocks on exp), and out-projection /
normalization work is deferred into a pending queue drained one
instruction per loop iteration to fill PE bubbles.
"""

import heapq
import sys
import types as _types
from collections import deque

import numpy as np
import ml_dtypes

import concourse.mybir as mybir
import concourse.tile as tile
from concourse import bacc
from concourse.bass import ts
from concourse.bass_utils import run_bass_kernel_spmd

# ---- problem constants (hardcoded per harness contract) ----
B, S, D, H = 4, 2048, 1024, 16
HD = D // H            # 64 head dim
HPC = H // 2           # 8 heads per core
FG = HPC * HD          # 512 features per head-group
NCORES = 8
NST = S // 128         # 16 s-tiles
NDT = D // 128         # 8 d-tiles
NSB = S // 512         # 4 s/i-blocks

F32 = mybir.dt.float32
BF16 = mybir.dt.bfloat16
EXP = mybir.ActivationFunctionType.Exp
BF = ml_dtypes.bfloat16

# DMA-crossbar transpose row fold: True -> transposed row r lands at
# partition r % 128, extra dim r // 128 ("(dt p)"); False -> r // NDT,
# r % NDT ("(p dt)"). Weight layouts below mirror this. Verified in sim.
XPOSE_PMINOR = True
DEBUG_DUMP = False  # add dbg_* outputs (qT/kT/vA/outT) to the module


def _install_ntff_hook():
    """run_bass_kernel_spmd(trace=True) under axon needs antenv.axon_hooks,
    absent in this image; shim it with the boot module's ctypes hook."""
    if "antenv.axon_hooks" in sys.modules:
        return
    try:
        from trn_agent_boot.trn_boot import _ntff_profile_via_ctypes
    except ImportError:
        return
    m = _types.ModuleType("antenv.axon_hooks")
    m.get_axon_ntff_profile_hook = lambda: _ntff_profile_via_ctypes(
        "/opt/axon/libaxon_pjrt.so"
    )
    m.set_axon_ntff_profile_hook = lambda h: None
    sys.modules["antenv.axon_hooks"] = m


def _w_fold():
    return "(dt p) f -> p dt f" if XPOSE_PMINOR else "(p dt) f -> p dt f"


def _body(tc, io):
    nc = tc.nc
    x, wq, wk, wv, wp = io["x"], io["wq"], io["wk"], io["wv"], io["wp"]
    cst_d, out = io["cst"], io["out"]

    x_sb = x.rearrange("(sb p) d -> sb p d", p=512)          # [4,512,1024]
    out_r = out.rearrange("(it p) e -> it p e", p=128)       # [16,128,1024]

    with tc.tile_pool(name="persist", bufs=1) as pp:
        xT = pp.tile([128, NDT, S], BF16, name="xT")         # [d, dt, s]
        qT = pp.tile([128, 4, S], BF16, name="qT")           # [f, pair, s]
        kT = pp.tile([128, 4, S], BF16, name="kT")
        vA = pp.tile([128, NST, HPC, HD + 2], BF16, name="vA")  # v | ones
        outT = pp.tile([128, 4, S], BF16, name="outT")       # [f, pair, i]
        wqt = pp.tile([128, NDT, FG], BF16, name="wqt")
        wkt = pp.tile([128, NDT, FG], BF16, name="wkt")
        wvt = pp.tile([128, NDT, FG], BF16, name="wvt")
        wpt = pp.tile([128, 4, D], BF16, name="wpt")
        cst = pp.tile([128, 264], BF16, name="cst")

        # first x chunk as four independent per-st crossbar transposes on
        # the sync queue; weights go via the scalar queue so same-queue DMA
        # chaining doesn't serialize the transposes behind them
        x_st = x.rearrange("(st p) d -> st p d", p=128)
        nc.scalar.dma_start_transpose(xT[:, :, ts(0, 128)], x_st[0])
        nc.scalar.dma_start_transpose(xT[:, :, ts(2, 128)], x_st[2])
        nc.sync.dma_start(out=wvt[:, 0:4, :], in_=wv[:, 0:4, :])
        nc.sync.dma_start_transpose(xT[:, :, ts(1, 128)], x_st[1])
        nc.sync.dma_start(out=wvt[:, 4:8, :], in_=wv[:, 4:8, :])
        nc.sync.dma_start_transpose(xT[:, :, ts(3, 128)], x_st[3])
        nc.sync.dma_start(out=wqt, in_=wq)
        nc.sync.dma_start(out=wkt, in_=wk)
        nc.sync.dma_start(out=cst, in_=cst_d)
        nc.sync.dma_start(out=wpt, in_=wp)
        # denominator columns of vA: even heads (attn@v half 0, stationary
        # window [0:65]) carry ones at col 64; odd heads (half 1, window
        # [0:66]) carry 0 at col 64 and ones at col 65, so half 1's
        # denominator row lands on partition 65 — lane-aligned with the
        # reciprocal input (no PSUM->SBUF partition-shift DMA needed)
        vA_r = vA.rearrange("p s (ht two) c -> p s ht two c", two=2)
        nc.vector.tensor_copy(
            vA_r[:, :, :, 0, 64:65],
            cst[:, 128:129].unsqueeze(1).unsqueeze(1)
            .to_broadcast([128, NST, 4, 1]),
        )
        nc.vector.tensor_copy(
            vA_r[:, :, :, 1, 64:66],
            cst[:, 129:131].unsqueeze(1).unsqueeze(1)
            .to_broadcast([128, NST, 4, 2]),
        )

        with (
            tc.tile_pool(name="psc", bufs=2, space="PSUM") as psc,
            tc.tile_pool(name="poa", bufs=1, space="PSUM") as poa,
            tc.tile_pool(name="psh", bufs=2, space="PSUM") as psh,
            tc.tile_pool(name="swork", bufs=2) as sw,
            tc.tile_pool(name="sat", bufs=4) as sat,
        ):
            pending = deque()
            delayed = []  # heap of (ready_slot, seq, fn)
            slot = [0]
            seq = [0]

            def pop(n=1, tile_starts=True):
                slot[0] += 1
                while delayed and delayed[0][0] <= slot[0]:
                    pending.append(heapq.heappop(delayed)[2])
                for _ in range(min(n, len(pending))):
                    if pending[0][0] and not tile_starts:
                        # a proj-tile start holds both shared-ring PSUM
                        # banks for ~8 pops; during phase A those banks
                        # cycle the v/qk chains, so don't start one here
                        return
                    pending.popleft()[1]()

            def defer(fn, delay, starts_tile=False):
                seq[0] += 1
                heapq.heappush(delayed,
                               (slot[0] + delay, seq[0], (starts_tile, fn)))

            def drain():
                while delayed or pending:
                    pop(1)

            def proj_tile(it):
                st = {}

                def mk(ct, et):
                    def f():
                        if "p" not in st:
                            st["p"] = [
                                psh.tile([128, 512], F32, name=f"pres{j}",
                                         tag="sh")
                                for j in range(2)
                            ]
                        nc.tensor.matmul(
                            st["p"][et], outT[:, ct, ts(it, 128)],
                            wpt[:, ct, ts(et, 512)],
                            start=(ct == 0), stop=(ct == 3),
                        )
                        if ct == 3 and et == 1:
                            res = sw.tile([128, 2, 512], F32, name="res",
                                          tag="res")
                            nc.vector.tensor_copy(res[:, 0, :], st["p"][0])
                            nc.vector.tensor_copy(res[:, 1, :], st["p"][1])
                            nc.gpsimd.dma_start(out=out_r[it], in_=res)
                    return f

                return [mk(ct, et) for ct in range(4) for et in range(2)]

            def phase_a(sb):
                for st4 in range(4):
                    st_ = sb * 4 + st4
                    pv = psh.tile([128, 512], F32, name="pv", tag="sh")
                    for dt_ in range(NDT):
                        nc.tensor.matmul(
                            pv, xT[:, dt_, ts(st_, 128)], wvt[:, dt_, :],
                            start=(dt_ == 0), stop=(dt_ == NDT - 1),
                        )
                    nc.scalar.copy(
                        vA[:, st_, :, 0:HD],
                        pv.rearrange("p (h c) -> p h c", h=HPC),
                    )
                    pop(1, tile_starts=False)
                for p in range(4):
                    for wt_, dst in ((wqt, qT), (wkt, kT)):
                        pqk = psh.tile([128, 512], F32, name="pqk", tag="sh")
                        for dt_ in range(NDT):
                            nc.tensor.matmul(
                                pqk, wt_[:, dt_, ts(p, 128)],
                                xT[:, dt_, ts(sb, 512)],
                                start=(dt_ == 0), stop=(dt_ == NDT - 1),
                            )
                        nc.scalar.copy(dst[:, p, ts(sb, 512)], pqk)
                        pop(1, tile_starts=False)

            def attention(ib):
                njt = 4 * (ib + 1)
                # per-ib denominator pack at partitions 32p/32p+1: cols
                # [1024:1536) raw dens (DMA'd from each block's rr rows),
                # [0:512) reciprocals; dpb holds the bf16 recips
                dpk = sw.tile([128, 1536], F32, name="dpk", tag="dpk")
                dpb = sw.tile([128, 512], BF16, name="dpb", tag="dpb")
                dpk_r = dpk.rearrange("(g r) f -> g r f", r=32)
                dpb_r = dpb.rearrange("(g r) f -> g r f", r=32)
                norm_fins = []
                # diagonal j-tiles first: the accumulation group's start
                # (jt=4ib, full width) and stop (last off-diag, full width)
                # matmuls cover the whole PSUM region, so partial diagonal
                # tiles can be trimmed to cols [off:) — their skipped at2
                # columns are never written or read.
                jt_seq = list(range(4 * ib, njt)) + list(range(4 * ib))
                for p in range(4):
                    if p == 1 and ib + 1 < NSB:
                        # prefetch next x chunk transpose (runs during the
                        # attention phase where xT isn't being read, keeping
                        # the crossbar DMA off phase A's SBUF-hungry chains)
                        nc.sync.dma_start_transpose(
                            xT[:, :, ts(ib + 1, 512)], x_sb[ib + 1])
                    oa0 = poa.tile([HD + 1, 512], F32, name="oa0", tag="oa0")
                    oa1 = poa.tile([HD + 2, 512], F32, name="oa1", tag="oa1")
                    oa = (oa0, oa1)

                    def emit_av(jt, at2, off, start, stop):
                        for half in range(2):
                            nc.tensor.matmul(
                                oa[half][:, off:],
                                vA[:, jt, 2 * p + half, 0 : HD + 1 + half],
                                at2[:, half, off:],
                                start=start, stop=stop,
                                # ib=0's stop av is partial-width (the
                                # accumulation group start covers the full
                                # region)
                                skip_group_check=(ib == 0),
                            )

                    prev = None
                    for i, jt in enumerate(jt_seq):
                        off = max(0, (jt - ib * 4) * 128)
                        sc = psc.tile([128, 2, 512], F32, name="sc", tag="sc")
                        for half in range(2):
                            h0 = 64 * half
                            nc.tensor.matmul(
                                sc[:, half, off:],
                                kT[h0 : h0 + 64, p, ts(jt, 128)],
                                qT[h0 : h0 + 64, p,
                                   ib * 512 + off : (ib + 1) * 512],
                                start=True, stop=True,
                            )
                        at2 = sat.tile([128, 2, 512], BF16, name="at2",
                                       tag="at", bufs=6)
                        nc.scalar.activation(
                            at2[:, :, off:], sc[:, :, off:], EXP)
                        if jt >= ib * 4:
                            # causal mask: zero exp(score) where j > i
                            nc.gpsimd.affine_select(
                                out=at2[:, :, off:], in_=at2[:, :, off:],
                                compare_op=mybir.AluOpType.is_ge,
                                fill=0.0, base=0,
                                pattern=[[0, 2], [1, 512 - off]],
                                channel_multiplier=-1,
                            )
                        if DEBUG_DUMP and ib == 0:
                            nc.sync.dma_start(out=io["dbg_at2"][p, jt],
                                              in_=at2)
                        if prev is not None:
                            emit_av(*prev, start=(prev[0] == jt_seq[0]),
                                    stop=False)
                        prev = (jt, at2, off)
                        if ib < 2 or i % 2 == 0 or i >= njt - 2:
                            pop(1)
                    emit_av(*prev, start=(prev[0] == jt_seq[0]),
                            stop=True)

                    # normalization: copy accumulators off PSUM, invert the
                    # denominators (fast approx), broadcast via a K=2 PE
                    # matmul, scale into outT. The PE matmul + final mul are
                    # deferred (popped later) so the in-order PE queue never
                    # waits on the DVE recip latency.
                    oc = sw.tile([128, 512], F32, name="oc", tag="oc",
                                 bufs=5)
                    oc1t = sw.tile([64, 512], F32, name="oc1t",
                                   tag="oc1", bufs=5)
                    rr = sw.tile([66, 2, 512], F32, name="rr",
                                 tag="rr", bufs=3)
                    # denominator rows first: the 3.3us reciprocal is the
                    # long pole of this chain. (engines can't start at
                    # partition 65: copy oa1's [dummy, den1] rows, then den0
                    # over the dummy)
                    nc.vector.tensor_copy(rr[64:66, 0, :], oa1[64:66, :])
                    nc.vector.tensor_copy(rr[64:65, 0, :], oa0[64:65, :])
                    dg, dr0 = ((0, 0), (0, 2), (1, 0), (2, 0))[p]
                    nc.sync.dma_start(
                        out=dpk_r[dg, dr0 : dr0 + 2, 1024:1536],
                        in_=rr[64:66, 0, :])
                    nc.vector.tensor_copy(oc[0:64, :], oa0[0:64, :])
                    nc.vector.tensor_copy(oc1t, oa1[0:64, :])
                    # partition shift for half 1 (SBUF->SBUF DMA)
                    nc.sync.dma_start(out=oc[64:128, :], in_=oc1t)
                    if DEBUG_DUMP:
                        nc.sync.dma_start(out=io["dbg_den"][p, ib],
                                          in_=rr[64:66, :, :])
                        nc.sync.dma_start(out=io["dbg_oc"][p, ib], in_=oc)

                    def norm_fin(p=p, ib=ib, oc=oc):
                        prep = psh.tile([128, 512], F32, name="prep",
                                        tag="sh")
                        if p < 2:
                            c0 = 0 if p == 0 else 132
                            nc.tensor.matmul(
                                prep, cst[0:4, c0 : c0 + 128],
                                dpb_r[0, 0:4, :], start=True, stop=True)
                        else:
                            b_ = 32 * (p - 1)
                            nc.tensor.matmul(
                                prep, cst[b_ : b_ + 2, 0:128],
                                dpb_r[p - 1, 0:2, :], start=True, stop=True)
                        nc.vector.tensor_mul(outT[:, p, ts(ib, 512)], oc,
                                             prep)

                    # the norm chain (copies -> shift DMAs -> reciprocal ->
                    # cast) takes ~6us on DVE/sync; pop the PE-side finish
                    # only after enough slots that it never blocks the PE
                    norm_fins.append(norm_fin)
                    # batched inversion, emitted as each group's dens land
                    # (p0+p1 @ base 0, p2 @ 32, p3 @ 64) so only p3's
                    # reciprocal remains on the kernel's tail
                    rsl = {1: (0, 4), 2: (1, 2), 3: (2, 2)}.get(p)
                    if rsl is not None:
                        g_, nr = rsl
                        nc.vector.reciprocal(dpk_r[g_, 0:nr, 0:512],
                                             dpk_r[g_, 0:nr, 1024:1536])
                        nc.vector.tensor_copy(dpb_r[g_, 0:nr, :],
                                              dpk_r[g_, 0:nr, 0:512])
                for nf in norm_fins:
                    defer(nf, 8)
                norm_fins.clear()

                for it in range(ib * 4, ib * 4 + 4):
                    for k_, c in enumerate(proj_tile(it)):
                        defer(c, 10, starts_tile=(k_ == 0))

            for sb in range(NSB):
                phase_a(sb)
                attention(sb)
            drain()
            if DEBUG_DUMP:
                nc.sync.dma_start(out=io["dbg_qT"], in_=qT)
                nc.sync.dma_start(out=io["dbg_kT"], in_=kT)
                nc.sync.dma_start(out=io["dbg_vA"], in_=vA)
                nc.sync.dma_start(out=io["dbg_outT"], in_=outT)


def build():
    nc = bacc.Bacc("TRN2", target_bir_lowering=False, debug=False,
                   num_devices=NCORES)
    io = {
        "x": nc.dram_tensor("x", [S, D], BF16, kind="ExternalInput").ap(),
        "wq": nc.dram_tensor("wq", [128, NDT, FG], BF16,
                             kind="ExternalInput").ap(),
        "wk": nc.dram_tensor("wk", [128, NDT, FG], BF16,
                             kind="ExternalInput").ap(),
        "wv": nc.dram_tensor("wv", [128, NDT, FG], BF16,
                             kind="ExternalInput").ap(),
        "wp": nc.dram_tensor("wp", [128, 4, D], BF16,
                             kind="ExternalInput").ap(),
        "cst": nc.dram_tensor("cst", [128, 264], BF16,
                              kind="ExternalInput").ap(),
        "out": nc.dram_tensor("out", [S, D], F32, kind="ExternalOutput").ap(),
    }
    if DEBUG_DUMP:
        io["dbg_qT"] = nc.dram_tensor("dbg_qT", [128, 4, S], BF16,
                                      kind="ExternalOutput").ap()
        io["dbg_kT"] = nc.dram_tensor("dbg_kT", [128, 4, S], BF16,
                                      kind="ExternalOutput").ap()
        io["dbg_vA"] = nc.dram_tensor("dbg_vA", [128, NST, HPC, HD + 2],
                                      BF16, kind="ExternalOutput").ap()
        io["dbg_outT"] = nc.dram_tensor("dbg_outT", [128, 4, S], BF16,
                                        kind="ExternalOutput").ap()
        io["dbg_den"] = nc.dram_tensor("dbg_den", [4, 4, 2, 2, 512], F32,
                                       kind="ExternalOutput").ap()
        io["dbg_oc"] = nc.dram_tensor("dbg_oc", [4, 4, 128, 512], F32,
                                      kind="ExternalOutput").ap()
        io["dbg_at2"] = nc.dram_tensor("dbg_at2", [4, 4, 128, 2, 512], BF16,
                                       kind="ExternalOutput").ap()
    with tile.TileContext(nc) as tc:
        _body(tc, io)
    nc.compile()
    return nc


def _make_cst():
    cst = np.zeros((128, 264), dtype=BF)
    # K=4 broadcast stationaries. Pair p's denominators live at partitions
    # 32*(p//2) + 2*(p%2) + {0,1}; matmul operand base partitions must be
    # 0/32/64, so pairs share a base in twos and select their own rows via
    # a column block (the other pair's rows are zero there):
    #   cols [0:128)   -> even pairs (rows base+0/+1 carry the patterns)
    #   cols [132:260) -> odd pairs (rows base+2/+3)
    cst[0, 0:64] = BF(1.0)      # p0 (K=4 @ base 0, cols 0:128)
    cst[1, 64:128] = BF(1.0)
    cst[2, 132:196] = BF(1.0)   # p1 (K=4 @ base 0, cols 132:260)
    cst[3, 196:260] = BF(1.0)
    cst[32, 0:64] = BF(1.0)     # p2 (K=2 @ base 32)
    cst[33, 64:128] = BF(1.0)
    cst[64, 0:64] = BF(1.0)     # p3 (K=2 @ base 64)
    cst[65, 64:128] = BF(1.0)
    cst[:, 128] = BF(1.0)  # ones for vA denominator columns
    cst[:, 129] = BF(0.0)
    cst[:, 130] = BF(1.0)
    return cst


def _fold_w(w):
    # [D, F] -> [128, D//128, F]: partition p holds rows d = dt*128 + p
    return np.ascontiguousarray(
        w.reshape(w.shape[0] // 128, 128, w.shape[1]).transpose(1, 0, 2)
    ).astype(BF)


def _host_inputs(x, W_attn, b_attn, W_proj):
    assert not np.any(b_attn), "kernel assumes b_attn == 0 (spec fill: zeros)"
    cst = _make_cst()
    in_maps = []
    for c in range(NCORES):
        b, g = divmod(c, 2)
        in_maps.append({
            "x": np.ascontiguousarray(x[b]).astype(BF),
            # fold the 1/sqrt(HD) score scale into wq (exact: * 2^-3)
            "wq": _fold_w(W_attn[:, g * FG : (g + 1) * FG]
                          * np.float32(0.125)),
            "wk": _fold_w(W_attn[:, D + g * FG : D + (g + 1) * FG]),
            "wv": _fold_w(W_attn[:, 2 * D + g * FG : 2 * D + (g + 1) * FG]),
            "wp": _fold_w(W_proj[g * FG : (g + 1) * FG, :]),
            "cst": cst,
        })
    return in_maps


_NC_CACHE = {}


def kernel(x, W_attn, b_attn, W_proj, b_proj, _trace=False):
    x = np.asarray(x)
    W_attn = np.asarray(W_attn)
    b_attn = np.asarray(b_attn)
    W_proj = np.asarray(W_proj)
    b_proj = np.asarray(b_proj)

    if "nc" not in _NC_CACHE:
        _NC_CACHE["nc"] = build()
    nc = _NC_CACHE["nc"]

    in_maps = _host_inputs(x, W_attn, b_attn, W_proj)
    kwargs = {}
    if _trace:
        _install_ntff_hook()
        kwargs = dict(trace=True, trace_cores=[0])
    res = run_bass_kernel_spmd(nc, in_maps, core_ids=list(range(NCORES)),
                               **kwargs)
    y = np.empty((B, S, D), dtype=np.float32)
    for b in range(B):
        y[b] = (res.results[2 * b]["out"] + res.results[2 * b + 1]["out"]
                + b_proj.astype(np.float32))
    if _trace:
        kernel.last_exec_time_ns = res.exec_time_ns
        kernel.last_trace = res.instructions_and_trace
    return y


# revision 50
# speedup vs baseline: 1.0081x; 1.0081x over previous
"""Causal multi-head attention block on 8 Trainium2 NeuronCores.

Problem: x[4,2048,1024] -> qkv proj -> 16-head causal attention -> out proj.

Sharding: 8 cores = 4 batches x 2 head-groups (8 heads each). Each core
computes, for its (batch, head-group):
  - xT (feature-on-partition) via DMA-crossbar transpose of bf16 x
  - qT/kT (feature-on-partition) and v (natural layout), all bf16
  - causal attention with scores computed transposed (scoresT[j, i]):
    fp32 PSUM scores -> exp on the Act engine (bf16 out, fully-masked
    columns skipped), causal zero-fill on GpSimd, row-sums via an
    appended ones-column on v in the attn@v matmul
  - softmax denominators inverted with reciprocal_approx_fast and
    broadcast to 64 partitions with a tiny K=2 PE matmul
  - partial out-projection with its 512 rows of W_proj
Host sums the two partials per batch and adds b_proj.

All matmuls run in bf16 (1 cycle/row on HW vs ~2 for f32r; fp32 PSUM
accumulation). The emission order software-pipelines the attention inner
loop (attn@v for tile jt-1 is emitted after scores for jt so the PE
in-order queue never head-of-line blocks on exp), and out-projection /
normalization work is deferred into a pending queue drained one
instruction per loop iteration to fill PE bubbles.
"""

import heapq
import sys
import types as _types
from collections import deque

import numpy as np
import ml_dtypes

import concourse.mybir as mybir
import concourse.tile as tile
from concourse import bacc
from concourse.bass import ts
from concourse.bass_utils import run_bass_kernel_spmd

# ---- problem constants (hardcoded per harness contract) ----
B, S, D, H = 4, 2048, 1024, 16
HD = D // H            # 64 head dim
HPC = H // 2           # 8 heads per core
FG = HPC * HD          # 512 features per head-group
NCORES = 8
NST = S // 128         # 16 s-tiles
NDT = D // 128         # 8 d-tiles
NSB = S // 512         # 4 s/i-blocks

F32 = mybir.dt.float32
BF16 = mybir.dt.bfloat16
EXP = mybir.ActivationFunctionType.Exp
BF = ml_dtypes.bfloat16

# DMA-crossbar transpose row fold: True -> transposed row r lands at
# partition r % 128, extra dim r // 128 ("(dt p)"); False -> r // NDT,
# r % NDT ("(p dt)"). Weight layouts below mirror this. Verified in sim.
XPOSE_PMINOR = True
DEBUG_DUMP = False  # add dbg_* outputs (qT/kT/vA/outT) to the module


def _install_ntff_hook():
    """run_bass_kernel_spmd(trace=True) under axon needs antenv.axon_hooks,
    absent in this image; shim it with the boot module's ctypes hook."""
    if "antenv.axon_hooks" in sys.modules:
        return
    try:
        from trn_agent_boot.trn_boot import _ntff_profile_via_ctypes
    except ImportError:
        return
    m = _types.ModuleType("antenv.axon_hooks")
    m.get_axon_ntff_profile_hook = lambda: _ntff_profile_via_ctypes(
        "/opt/axon/libaxon_pjrt.so"
    )
    m.set_axon_ntff_profile_hook = lambda h: None
    sys.modules["antenv.axon_hooks"] = m


def _w_fold():
    return "(dt p) f -> p dt f" if XPOSE_PMINOR else "(p dt) f -> p dt f"


def _body(tc, io):
    nc = tc.nc
    x, wq, wk, wv, wp = io["x"], io["wq"], io["wk"], io["wv"], io["wp"]
    cst_d, out = io["cst"], io["out"]

    x_sb = x.rearrange("(sb p) d -> sb p d", p=512)          # [4,512,1024]
    out_r = out.rearrange("(it p) e -> it p e", p=128)       # [16,128,1024]

    with tc.tile_pool(name="persist", bufs=1) as pp:
        xT = pp.tile([128, NDT, S], BF16, name="xT")         # [d, dt, s]
        qT = pp.tile([128, 4, S], BF16, name="qT")           # [f, pair, s]
        kT = pp.tile([128, 4, S], BF16, name="kT")
        vA = pp.tile([128, NST, HPC, HD + 2], BF16, name="vA")  # v | ones
        outT = pp.tile([128, 4, S], BF16, name="outT")       # [f, pair, i]
        wqt = pp.tile([128, NDT, FG], BF16, name="wqt")
        wkt = pp.tile([128, NDT, FG], BF16, name="wkt")
        wvt = pp.tile([128, NDT, FG], BF16, name="wvt")
        wpt = pp.tile([128, 4, D], BF16, name="wpt")
        cst = pp.tile([128, 264], BF16, name="cst")

        # first x chunk as four independent per-st crossbar transposes on
        # the sync queue; weights go via the scalar queue so same-queue DMA
        # chaining doesn't serialize the transposes behind them
        x_st = x.rearrange("(st p) d -> st p d", p=128)
        nc.scalar.dma_start_transpose(xT[:, :, ts(0, 128)], x_st[0])
        nc.scalar.dma_start_transpose(xT[:, :, ts(2, 128)], x_st[2])
        nc.sync.dma_start(out=wvt[:, 0:4, :], in_=wv[:, 0:4, :])
        nc.sync.dma_start_transpose(xT[:, :, ts(1, 128)], x_st[1])
        nc.sync.dma_start(out=wvt[:, 4:8, :], in_=wv[:, 4:8, :])
        nc.sync.dma_start_transpose(xT[:, :, ts(3, 128)], x_st[3])
        nc.sync.dma_start(out=wqt, in_=wq)
        nc.sync.dma_start(out=wkt, in_=wk)
        nc.sync.dma_start(out=cst, in_=cst_d)
        nc.sync.dma_start(out=wpt, in_=wp)
        # denominator columns of vA: even heads (attn@v half 0, stationary
        # window [0:65]) carry ones at col 64; odd heads (half 1, window
        # [0:66]) carry 0 at col 64 and ones at col 65, so half 1's
        # denominator row lands on partition 65 — lane-aligned with the
        # reciprocal input (no PSUM->SBUF partition-shift DMA needed)
        vA_r = vA.rearrange("p s (ht two) c -> p s ht two c", two=2)
        nc.vector.tensor_copy(
            vA_r[:, :, :, 0, 64:65],
            cst[:, 128:129].unsqueeze(1).unsqueeze(1)
            .to_broadcast([128, NST, 4, 1]),
        )
        nc.vector.tensor_copy(
            vA_r[:, :, :, 1, 64:66],
            cst[:, 129:131].unsqueeze(1).unsqueeze(1)
            .to_broadcast([128, NST, 4, 2]),
        )

        with (
            tc.tile_pool(name="psc", bufs=2, space="PSUM") as psc,
            tc.tile_pool(name="poa", bufs=1, space="PSUM") as poa,
            tc.tile_pool(name="psh", bufs=2, space="PSUM") as psh,
            tc.tile_pool(name="swork", bufs=2) as sw,
            tc.tile_pool(name="sat", bufs=4) as sat,
        ):
            pending = deque()
            delayed = []  # heap of (ready_slot, seq, fn)
            slot = [0]
            seq = [0]

            def pop(n=1, tile_starts=True):
                slot[0] += 1
                while delayed and delayed[0][0] <= slot[0]:
                    pending.append(heapq.heappop(delayed)[2])
                for _ in range(min(n, len(pending))):
                    if pending[0][0] and not tile_starts:
                        # a proj-tile start holds both shared-ring PSUM
                        # banks for ~8 pops; during phase A those banks
                        # cycle the v/qk chains, so don't start one here
                        return
                    pending.popleft()[1]()

            def defer(fn, delay, starts_tile=False):
                seq[0] += 1
                heapq.heappush(delayed,
                               (slot[0] + delay, seq[0], (starts_tile, fn)))

            def drain():
                while delayed or pending:
                    pop(1)

            def proj_tile(it):
                st = {}

                def mk(ct, et):
                    def f():
                        if "p" not in st:
                            st["p"] = [
                                psh.tile([128, 512], F32, name=f"pres{j}",
                                         tag="sh")
                                for j in range(2)
                            ]
                        nc.tensor.matmul(
                            st["p"][et], outT[:, ct, ts(it, 128)],
                            wpt[:, ct, ts(et, 512)],
                            start=(ct == 0), stop=(ct == 3),
                        )
                        if ct == 3 and et == 1:
                            res = sw.tile([128, 2, 512], F32, name="res",
                                          tag="res")
                            nc.vector.tensor_copy(res[:, 0, :], st["p"][0])
                            nc.vector.tensor_copy(res[:, 1, :], st["p"][1])
                            nc.gpsimd.dma_start(out=out_r[it], in_=res)
                    return f

                return [mk(ct, et) for ct in range(4) for et in range(2)]

            def phase_a(sb):
                for st4 in range(4):
                    st_ = sb * 4 + st4
                    pv = psh.tile([128, 512], F32, name="pv", tag="sh")
                    for dt_ in range(NDT):
                        nc.tensor.matmul(
                            pv, xT[:, dt_, ts(st_, 128)], wvt[:, dt_, :],
                            start=(dt_ == 0), stop=(dt_ == NDT - 1),
                        )
                    nc.scalar.copy(
                        vA[:, st_, :, 0:HD],
                        pv.rearrange("p (h c) -> p h c", h=HPC),
                    )
                    pop(1, tile_starts=False)
                for p in range(4):
                    for wt_, dst in ((wqt, qT), (wkt, kT)):
                        pqk = psh.tile([128, 512], F32, name="pqk", tag="sh")
                        for dt_ in range(NDT):
                            nc.tensor.matmul(
                                pqk, wt_[:, dt_, ts(p, 128)],
                                xT[:, dt_, ts(sb, 512)],
                                start=(dt_ == 0), stop=(dt_ == NDT - 1),
                            )
                        nc.scalar.copy(dst[:, p, ts(sb, 512)], pqk)
                        pop(1, tile_starts=False)

            def attention(ib):
                njt = 4 * (ib + 1)
                # per-ib denominator pack at partitions 32p/32p+1: cols
                # [1024:1536) raw dens (DMA'd from each block's rr rows),
                # [0:512) reciprocals; dpb holds the bf16 recips
                dpk = sw.tile([128, 1536], F32, name="dpk", tag="dpk")
                dpb = sw.tile([128, 512], BF16, name="dpb", tag="dpb")
                dpk_r = dpk.rearrange("(g r) f -> g r f", r=32)
                dpb_r = dpb.rearrange("(g r) f -> g r f", r=32)
                norm_fins = []
                # diagonal j-tiles first: the accumulation group's start
                # (jt=4ib, full width) and stop (last off-diag, full width)
                # matmuls cover the whole PSUM region, so partial diagonal
                # tiles can be trimmed to cols [off:) — their skipped at2
                # columns are never written or read.
                jt_seq = list(range(4 * ib, njt)) + list(range(4 * ib))
                for p in range(4):
                    if p == 1 and ib + 1 < NSB:
                        # prefetch next x chunk transpose (runs during the
                        # attention phase where xT isn't being read, keeping
                        # the crossbar DMA off phase A's SBUF-hungry chains)
                        nc.sync.dma_start_transpose(
                            xT[:, :, ts(ib + 1, 512)], x_sb[ib + 1])
                    oa0 = poa.tile([HD + 1, 512], F32, name="oa0", tag="oa0")
                    oa1 = poa.tile([HD + 2, 512], F32, name="oa1", tag="oa1")
                    oa = (oa0, oa1)

                    def emit_av(jt, at2, off, start, stop):
                        for half in range(2):
                            nc.tensor.matmul(
                                oa[half][:, off:],
                                vA[:, jt, 2 * p + half, 0 : HD + 1 + half],
                                at2[:, half, off:],
                                start=start, stop=stop,
                                # ib=0's stop av is partial-width (the
                                # accumulation group start covers the full
                                # region)
                                skip_group_check=(ib == 0),
                            )

                    prev = None
                    for i, jt in enumerate(jt_seq):
                        off = max(0, (jt - ib * 4) * 128)
                        sc = psc.tile([128, 2, 512], F32, name="sc", tag="sc")
                        for half in range(2):
                            h0 = 64 * half
                            nc.tensor.matmul(
                                sc[:, half, off:],
                                kT[h0 : h0 + 64, p, ts(jt, 128)],
                                qT[h0 : h0 + 64, p,
                                   ib * 512 + off : (ib + 1) * 512],
                                start=True, stop=True,
                            )
                        at2 = sat.tile([128, 2, 512], BF16, name="at2",
                                       tag="at", bufs=8)
                        nc.scalar.activation(
                            at2[:, :, off:], sc[:, :, off:], EXP)
                        if jt >= ib * 4:
                            # causal mask: zero exp(score) where j > i
                            nc.gpsimd.affine_select(
                                out=at2[:, :, off:], in_=at2[:, :, off:],
                                compare_op=mybir.AluOpType.is_ge,
                                fill=0.0, base=0,
                                pattern=[[0, 2], [1, 512 - off]],
                                channel_multiplier=-1,
                            )
                        if DEBUG_DUMP and ib == 0:
                            nc.sync.dma_start(out=io["dbg_at2"][p, jt],
                                              in_=at2)
                        if prev is not None:
                            emit_av(*prev, start=(prev[0] == jt_seq[0]),
                                    stop=False)
                        prev = (jt, at2, off)
                        if ib < 2 or i % 2 == 0 or i >= njt - 2:
                            pop(1)
                    emit_av(*prev, start=(prev[0] == jt_seq[0]),
                            stop=True)

                    # normalization: copy accumulators off PSUM, invert the
                    # denominators (fast approx), broadcast via a K=2 PE
                    # matmul, scale into outT. The PE matmul + final mul are
                    # deferred (popped later) so the in-order PE queue never
                    # waits on the DVE recip latency.
                    oc = sw.tile([128, 512], F32, name="oc", tag="oc",
                                 bufs=5)
                    oc1t = sw.tile([64, 512], F32, name="oc1t",
                                   tag="oc1", bufs=5)
                    rr = sw.tile([66, 2, 512], F32, name="rr",
                                 tag="rr", bufs=3)
                    # denominator rows first: the 3.3us reciprocal is the
                    # long pole of this chain. (engines can't start at
                    # partition 65: copy oa1's [dummy, den1] rows, then den0
                    # over the dummy)
                    nc.vector.tensor_copy(rr[64:66, 0, :], oa1[64:66, :])
                    nc.vector.tensor_copy(rr[64:65, 0, :], oa0[64:65, :])
                    if p < 3:
                        dg, dr0 = ((0, 0), (0, 2), (1, 0))[p]
                        nc.sync.dma_start(
                            out=dpk_r[dg, dr0 : dr0 + 2, 1024:1536],
                            in_=rr[64:66, 0, :])
                    else:
                        # p3's dens already sit at partitions 64-65: invert
                        # in place on rr and cast straight into dpb
                        nc.vector.reciprocal(rr[64:66, 1, :],
                                             rr[64:66, 0, :])
                        nc.vector.tensor_copy(dpb_r[2, 0:2, :],
                                              rr[64:66, 1, :])
                    nc.vector.tensor_copy(oc[0:64, :], oa0[0:64, :])
                    nc.vector.tensor_copy(oc1t, oa1[0:64, :])
                    # partition shift for half 1 (SBUF->SBUF DMA)
                    nc.sync.dma_start(out=oc[64:128, :], in_=oc1t)
                    if DEBUG_DUMP:
                        nc.sync.dma_start(out=io["dbg_den"][p, ib],
                                          in_=rr[64:66, :, :])
                        nc.sync.dma_start(out=io["dbg_oc"][p, ib], in_=oc)

                    def norm_fin(p=p, ib=ib, oc=oc):
                        prep = psh.tile([128, 512], F32, name="prep",
                                        tag="sh")
                        if p < 2:
                            c0 = 0 if p == 0 else 132
                            nc.tensor.matmul(
                                prep, cst[0:4, c0 : c0 + 128],
                                dpb_r[0, 0:4, :], start=True, stop=True)
                        else:
                            b_ = 32 * (p - 1)
                            nc.tensor.matmul(
                                prep, cst[b_ : b_ + 2, 0:128],
                                dpb_r[p - 1, 0:2, :], start=True, stop=True)
                        nc.vector.tensor_mul(outT[:, p, ts(ib, 512)], oc,
                                             prep)

                    # the norm chain (copies -> shift DMAs -> reciprocal ->
                    # cast) takes ~6us on DVE/sync; pop the PE-side finish
                    # only after enough slots that it never blocks the PE
                    norm_fins.append(norm_fin)
                    # batched inversion, emitted as each group's dens land
                    # (p0+p1 @ base 0, p2 @ 32, p3 @ 64) so only p3's
                    # reciprocal remains on the kernel's tail
                    rsl = {1: (0, 4), 2: (1, 2)}.get(p)
                    if rsl is not None:
                        g_, nr = rsl
                        nc.vector.reciprocal(dpk_r[g_, 0:nr, 0:512],
                                             dpk_r[g_, 0:nr, 1024:1536])
                        nc.vector.tensor_copy(dpb_r[g_, 0:nr, :],
                                              dpk_r[g_, 0:nr, 0:512])
                for nf in norm_fins:
                    defer(nf, 8)
                norm_fins.clear()

                for it in range(ib * 4, ib * 4 + 4):
                    for k_, c in enumerate(proj_tile(it)):
                        defer(c, 10, starts_tile=(k_ == 0))

            for sb in range(NSB):
                phase_a(sb)
                attention(sb)
            drain()
            if DEBUG_DUMP:
                nc.sync.dma_start(out=io["dbg_qT"], in_=qT)
                nc.sync.dma_start(out=io["dbg_kT"], in_=kT)
                nc.sync.dma_start(out=io["dbg_vA"], in_=vA)
                nc.sync.dma_start(out=io["dbg_outT"], in_=outT)


def build():
    nc = bacc.Bacc("TRN2", target_bir_lowering=False, debug=False,
                   num_devices=NCORES)
    io = {
        "x": nc.dram_tensor("x", [S, D], BF16, kind="ExternalInput").ap(),
        "wq": nc.dram_tensor("wq", [128, NDT, FG], BF16,
                             kind="ExternalInput").ap(),
        "wk": nc.dram_tensor("wk", [128, NDT, FG], BF16,
                             kind="ExternalInput").ap(),
        "wv": nc.dram_tensor("wv", [128, NDT, FG], BF16,
                             kind="ExternalInput").ap(),
        "wp": nc.dram_tensor("wp", [128, 4, D], BF16,
                             kind="ExternalInput").ap(),
        "cst": nc.dram_tensor("cst", [128, 264], BF16,
                              kind="ExternalInput").ap(),
        "out": nc.dram_tensor("out", [S, D], F32, kind="ExternalOutput").ap(),
    }
    if DEBUG_DUMP:
        io["dbg_qT"] = nc.dram_tensor("dbg_qT", [128, 4, S], BF16,
                                      kind="ExternalOutput").ap()
        io["dbg_kT"] = nc.dram_tensor("dbg_kT", [128, 4, S], BF16,
                                      kind="ExternalOutput").ap()
        io["dbg_vA"] = nc.dram_tensor("dbg_vA", [128, NST, HPC, HD + 2],
                                      BF16, kind="ExternalOutput").ap()
        io["dbg_outT"] = nc.dram_tensor("dbg_outT", [128, 4, S], BF16,
                                        kind="ExternalOutput").ap()
        io["dbg_den"] = nc.dram_tensor("dbg_den", [4, 4, 2, 2, 512], F32,
                                       kind="ExternalOutput").ap()
        io["dbg_oc"] = nc.dram_tensor("dbg_oc", [4, 4, 128, 512], F32,
                                      kind="ExternalOutput").ap()
        io["dbg_at2"] = nc.dram_tensor("dbg_at2", [4, 4, 128, 2, 512], BF16,
                                       kind="ExternalOutput").ap()
    with tile.TileContext(nc) as tc:
        _body(tc, io)
    nc.compile()
    return nc


def _make_cst():
    cst = np.zeros((128, 264), dtype=BF)
    # K=4 broadcast stationaries. Pair p's denominators live at partitions
    # 32*(p//2) + 2*(p%2) + {0,1}; matmul operand base partitions must be
    # 0/32/64, so pairs share a base in twos and select their own rows via
    # a column block (the other pair's rows are zero there):
    #   cols [0:128)   -> even pairs (rows base+0/+1 carry the patterns)
    #   cols [132:260) -> odd pairs (rows base+2/+3)
    cst[0, 0:64] = BF(1.0)      # p0 (K=4 @ base 0, cols 0:128)
    cst[1, 64:128] = BF(1.0)
    cst[2, 132:196] = BF(1.0)   # p1 (K=4 @ base 0, cols 132:260)
    cst[3, 196:260] = BF(1.0)
    cst[32, 0:64] = BF(1.0)     # p2 (K=2 @ base 32)
    cst[33, 64:128] = BF(1.0)
    cst[64, 0:64] = BF(1.0)     # p3 (K=2 @ base 64)
    cst[65, 64:128] = BF(1.0)
    cst[:, 128] = BF(1.0)  # ones for vA denominator columns
    cst[:, 129] = BF(0.0)
    cst[:, 130] = BF(1.0)
    return cst


def _fold_w(w):
    # [D, F] -> [128, D//128, F]: partition p holds rows d = dt*128 + p
    return np.ascontiguousarray(
        w.reshape(w.shape[0] // 128, 128, w.shape[1]).transpose(1, 0, 2)
    ).astype(BF)


def _host_inputs(x, W_attn, b_attn, W_proj):
    assert not np.any(b_attn), "kernel assumes b_attn == 0 (spec fill: zeros)"
    cst = _make_cst()
    in_maps = []
    for c in range(NCORES):
        b, g = divmod(c, 2)
        in_maps.append({
            "x": np.ascontiguousarray(x[b]).astype(BF),
            # fold the 1/sqrt(HD) score scale into wq (exact: * 2^-3)
            "wq": _fold_w(W_attn[:, g * FG : (g + 1) * FG]
                          * np.float32(0.125)),
            "wk": _fold_w(W_attn[:, D + g * FG : D + (g + 1) * FG]),
            "wv": _fold_w(W_attn[:, 2 * D + g * FG : 2 * D + (g + 1) * FG]),
            "wp": _fold_w(W_proj[g * FG : (g + 1) * FG, :]),
            "cst": cst,
        })
    return in_maps


_NC_CACHE = {}


def kernel(x, W_attn, b_attn, W_proj, b_proj, _trace=False):
    x = np.asarray(x)
    W_attn = np.asarray(W_attn)
    b_attn = np.asarray(b_attn)
    W_proj = np.asarray(W_proj)
    b_proj = np.asarray(b_proj)

    if "nc" not in _NC_CACHE:
        _NC_CACHE["nc"] = build()
    nc = _NC_CACHE["nc"]

    in_maps = _host_inputs(x, W_attn, b_attn, W_proj)
    kwargs = {}
    if _trace:
        _install_ntff_hook()
        kwargs = dict(trace=True, trace_cores=[0])
    res = run_bass_kernel_spmd(nc, in_maps, core_ids=list(range(NCORES)),
                               **kwargs)
    y = np.empty((B, S, D), dtype=np.float32)
    for b in range(B):
        y[b] = (res.results[2 * b]["out"] + res.results[2 * b + 1]["out"]
                + b_proj.astype(np.float32))
    if _trace:
        kernel.last_exec_time_ns = res.exec_time_ns
        kernel.last_trace = res.instructions_and_trace
    return y
